# revision 9
# baseline (speedup 1.0000x reference)
"""Trainium2 Bass kernel for nn_CAM_41377714929724 (CAM cross-attention module).

  a1  = f1 @ W                      [B,S,D]
  cc  = a1 @ f2^T                   [B,S,S]
  aatt = softmax(cc, axis=s)        (over rows)
  vatt = softmax(cc, axis=t).T      (over cols, transposed)
  out1 = (f1 @ aatt).swap(1,2)      [B,S,S]
  out2 = (f2 @ vatt).swap(1,2)      [B,S,S]

Sharding: pure data parallelism, 2 batches per core on 8 cores; W replicated.

Per core/batch dataflow (everything 1024x1024, fp32r matmuls @ full PE rate):
  a1T[e,s] = sum_d W[d,e] f1T[d,s]          (lhsT=W,    rhs=f1T)
  cc [s,t] = sum_e a1T[e,s] f2T[e,t]        (lhsT=a1T,  rhs=f2T)
  ccT[t,s] = sum_e f2T[e,t] a1T[e,s]        (lhsT=f2T,  rhs=a1T)
  amax[t]  = max_s cc  (gpsimd partition allreduce over cc tiles)
  vmax[s]  = max_t cc  (gpsimd partition allreduce over ccT tiles)
  e1 [u,t] = exp(cc  - amax[t])   in place  (DVE sub + ACT exp)
  e2T[t,s] = exp(ccT - vmax[s])   in place
  asum[t]  = sum_u e1   (gpsimd allreduce add) -> 1/asum per-partition via DRAM bounce
  vsum[s]  = sum_u e2T  (gpsimd allreduce add) -> 1/vsum
  out1[x,s] = (sum_u e1[u,x] f1T[u,s]) / asum[x]   (scale fused into PSUM drain)
  out2[s,t] = (sum_u e2T[u,s] f2T[u,t]) / vsum[s]

Column-halved softmax stats let ret-matmuls start while the other half is
still in flight, keeping the PE dense.
"""

import numpy as np
from contextlib import ExitStack

import concourse.bass as bass
import concourse.tile as tile
from concourse import bacc, mybir, bass_isa
from concourse.bass_utils import run_bass_kernel_spmd

f32 = mybir.dt.float32
f32r = mybir.dt.float32r

P = 128
N = 1024
NT = N // P          # 8 tiles per matrix dim
NB = 2               # batches per core
NCORES = 8
HALF = 512           # matmul moving free dim / psum bank


def _build():
    nc = bacc.Bacc("TRN2", target_bir_lowering=False, debug=False, num_devices=NCORES)

    f1t_d = nc.dram_tensor("f1t", [NB, N, N], f32r, kind="ExternalInput").ap()
    f2t_d = nc.dram_tensor("f2t", [NB, N, N], f32r, kind="ExternalInput").ap()
    w_d = nc.dram_tensor("w", [N, N], f32r, kind="ExternalInput").ap()
    o1_d = nc.dram_tensor("o1", [NB, N, N], f32, kind="ExternalOutput").ap()
    o2_d = nc.dram_tensor("o2", [NB, N, N], f32, kind="ExternalOutput").ap()

    with tile.TileContext(nc) as tc, ExitStack() as ctx:
        wp = ctx.enter_context(tc.tile_pool(name="wp", bufs=1))
        f1p = ctx.enter_context(tc.tile_pool(name="f1p", bufs=1))
        f2p = ctx.enter_context(tc.tile_pool(name="f2p", bufs=1))
        a1p = ctx.enter_context(tc.tile_pool(name="a1p", bufs=1))
        ccp = ctx.enter_context(tc.tile_pool(name="ccp", bufs=1))
        cctp = ctx.enter_context(tc.tile_pool(name="cctp", bufs=1))
        statp = ctx.enter_context(tc.tile_pool(name="statp", bufs=1))
        tmpp = ctx.enter_context(tc.tile_pool(name="tmpp", bufs=2))
        smallp = ctx.enter_context(tc.tile_pool(name="smallp", bufs=1))
        oretp = ctx.enter_context(tc.tile_pool(name="oretp", bufs=2))
        psp = ctx.enter_context(tc.tile_pool(name="psp", bufs=8, space="PSUM"))
        dscrp = ctx.enter_context(tc.tile_pool(name="dscrp", bufs=4, space="DRAM"))

        for b in range(NB):
            # ---- loads -------------------------------------------------
            ws, f1s, f2s = [], [], []
            for k in range(NT):
                wk = wp.tile([P, N], f32r, name=f"w{b}_{k}", tag=f"w{k}")
                nc.sync.dma_start(wk[:], w_d[k * P:(k + 1) * P, :])
                ws.append(wk)
            for k in range(NT):
                f1k = f1p.tile([P, N], f32r, name=f"f1_{b}_{k}", tag=f"f1{k}")
                nc.sync.dma_start(f1k[:], f1t_d[b, k * P:(k + 1) * P, :])
                f1s.append(f1k)
            for k in range(NT):
                f2k = f2p.tile([P, N], f32r, name=f"f2_{b}_{k}", tag=f"f2{k}")
                nc.sync.dma_start(f2k[:], f2t_d[b, k * P:(k + 1) * P, :])
                f2s.append(f2k)

            def mm1024(lhs_tiles, rhs_tiles, drain, m_range=range(NT), tagpfx="mm"):
                # out[m*128.., n*512..] = sum_k lhs_tiles[k][:,m*128..].T @ rhs_tiles[k][:,n*512..]
                for m in m_range:
                    for n in range(2):
                        ps = psp.tile([P, HALF], f32, name=f"ps_{tagpfx}", tag="ps")
                        for k in range(NT):
                            nc.tensor.matmul(
                                ps[:],
                                lhs_tiles[k][:, m * P:(m + 1) * P],
                                rhs_tiles[k][:, n * HALF:(n + 1) * HALF],
                                start=(k == 0),
                                stop=(k == NT - 1),
                            )
                        drain(m, n, ps)

            # ---- a1T = W.T-contract f1T : [e, s] ------------------------
            a1s = [a1p.tile([P, N], f32r, name=f"a1_{b}_{m}", tag=f"a1{m}")
                   for m in range(NT)]
            mm1024(ws, f1s,
                   lambda m, n, ps: nc.scalar.copy(
                       a1s[m][:, n * HALF:(n + 1) * HALF], ps[:]),
                   tagpfx="a1")

            # ---- cc[s,t] and ccT[t,s] -----------------------------------
            ccs = [ccp.tile([P, N], f32r, name=f"cc_{b}_{m}", tag=f"cc{m}")
                   for m in range(NT)]
            mm1024(a1s, f2s,
                   lambda m, n, ps: nc.vector.tensor_copy(
                       ccs[m][:, n * HALF:(n + 1) * HALF], ps[:]),
                   tagpfx="cc")
            ccts = [cctp.tile([P, N], f32r, name=f"cct_{b}_{m}", tag=f"cct{m}")
                    for m in range(NT)]
            mm1024(f2s, a1s,
                   lambda m, n, ps: nc.vector.tensor_copy(
                       ccts[m][:, n * HALF:(n + 1) * HALF], ps[:]),
                   tagpfx="cct")

            # ---- partition stats, per column half -----------------------
            # amax[t] = max_s cc[s, t]; vmax[s] = max_t cc[s, t]
            def colreduce(tiles, h, op, outtile, tagpfx):
                sl = slice(h * HALF, (h + 1) * HALF)
                for m in range(NT):
                    t = tmpp.tile([P, HALF], f32, name=f"t_{tagpfx}", tag="redtmp")
                    nc.gpsimd.partition_all_reduce(
                        t[:], tiles[m][:, sl].bitcast(f32), channels=P, reduce_op=op)
                    if m == 0:
                        nc.vector.tensor_copy(outtile[:], t[:])
                    else:
                        nc.vector.tensor_tensor(
                            out=outtile[:], in0=outtile[:], in1=t[:],
                            op=(mybir.AluOpType.max
                                if op == bass_isa.ReduceOp.max
                                else mybir.AluOpType.add))

            def exp_half(tiles, maxh, h):
                # tiles[:, half] = exp(tiles[:, half] - maxh), out f32r
                sl = slice(h * HALF, (h + 1) * HALF)
                for m in range(NT):
                    nc.vector.tensor_tensor(
                        out=tiles[m][:, sl], in0=tiles[m][:, sl].bitcast(f32),
                        in1=maxh[:], op=mybir.AluOpType.subtract)
                    nc.scalar.activation(
                        tiles[m][:, sl], tiles[m][:, sl].bitcast(f32),
                        mybir.ActivationFunctionType.Exp)

            # recip tiles: rsa[p, m] = 1/asum[m*128+p]
            rsa = smallp.tile([P, NT], f32, name=f"rsa{b}", tag="rsa")
            rsv = smallp.tile([P, NT], f32, name=f"rsv{b}", tag="rsv")

            for h in range(2):
                amaxh = statp.tile([P, HALF], f32, name=f"amaxh{b}{h}", tag="amaxh")
                colreduce(ccs, h, bass_isa.ReduceOp.max, amaxh, f"am{b}{h}")
                vmaxh = statp.tile([P, HALF], f32, name=f"vmaxh{b}{h}", tag="vmaxh")
                colreduce(ccts, h, bass_isa.ReduceOp.max, vmaxh, f"vm{b}{h}")
                exp_half(ccs, amaxh, h)    # -> e1 half
                exp_half(ccts, vmaxh, h)   # -> e2T half
                for name, tiles, rs in (("a", ccs, rsa), ("v", ccts, rsv)):
                    sumh = statp.tile([P, HALF], f32, name=f"sumh{name}{b}{h}",
                                      tag="sumh")
                    colreduce(tiles, h, bass_isa.ReduceOp.add, sumh, f"{name}s{b}{h}")
                    scr = dscrp.tile([1, HALF], f32, name=f"scr{name}{b}{h}",
                                     tag=f"scr{name}{h}")
                    nc.sync.dma_start(scr[:], sumh[0:1, :])
                    back = scr[:].rearrange("one (m p) -> (one p) m", p=P)
                    nc.sync.dma_start(rs[:, 4 * h:4 * h + 4], back)
                nc.vector.reciprocal(rsa[:, 4 * h:4 * h + 4], rsa[:, 4 * h:4 * h + 4])
                nc.vector.reciprocal(rsv[:, 4 * h:4 * h + 4], rsv[:, 4 * h:4 * h + 4])

            # ---- out1[x,s] = (e1.T-contract f1T) / asum[x] ---------------
            e1s = ccs
            e2s = ccts

            def ret_drain(out_d, rs):
                def d(m, n, ps):
                    ot = oretp.tile([P, HALF], f32, name="oret", tag="oret")
                    nc.scalar.activation(
                        ot[:], ps[:], mybir.ActivationFunctionType.Copy,
                        bias=0.0, scale=rs[:, m:m + 1])
                    nc.sync.dma_start(
                        out_d[b, m * P:(m + 1) * P, n * HALF:(n + 1) * HALF], ot[:])
                return d

            # m-tiles 0-3 depend only on column-half 0 stats; 4-7 on half 1
            mm1024(e1s, f1s, ret_drain(o1_d, rsa), m_range=range(0, 4), tagpfx="r1a")
            mm1024(e1s, f1s, ret_drain(o1_d, rsa), m_range=range(4, 8), tagpfx="r1b")
            mm1024(e2s, f2s, ret_drain(o2_d, rsv), m_range=range(0, 4), tagpfx="r2a")
            mm1024(e2s, f2s, ret_drain(o2_d, rsv), m_range=range(4, 8), tagpfx="r2b")

    nc.compile()
    return nc


_NC = None
TRACE = False
LAST = None


def _get_nc():
    global _NC
    if _NC is None:
        _NC = _build()
    return _NC


def kernel(f1_norm, f2_norm, corr_weights):
    f1_norm = np.ascontiguousarray(f1_norm, dtype=np.float32)
    f2_norm = np.ascontiguousarray(f2_norm, dtype=np.float32)
    w = np.ascontiguousarray(corr_weights, dtype=np.float32)
    B = f1_norm.shape[0]
    assert B == NB * NCORES

    # host-side feature-major transposes: f1t[b] = f1[b].T
    f1t = np.ascontiguousarray(np.swapaxes(f1_norm, 1, 2))
    f2t = np.ascontiguousarray(np.swapaxes(f2_norm, 1, 2))

    nc = _get_nc()
    in_maps = [
        {"f1t": f1t[c * NB:(c + 1) * NB], "f2t": f2t[c * NB:(c + 1) * NB], "w": w}
        for c in range(NCORES)
    ]
    res = run_bass_kernel_spmd(nc, in_maps, core_ids=list(range(NCORES)), trace=TRACE)
    global LAST
    LAST = res
    out1 = np.concatenate([res.results[c]["o1"] for c in range(NCORES)], axis=0)
    out2 = np.concatenate([res.results[c]["o2"] for c in range(NCORES)], axis=0)
    return out1, out2


# revision 15
# speedup vs baseline: 1.6333x; 1.6333x over previous
"""Trainium2 Bass kernel for nn_CAM_41377714929724 (CAM cross-attention module).

  a1  = f1 @ W                      [B,S,D]
  cc  = a1 @ f2^T                   [B,S,S]
  aatt = softmax(cc, axis=s)        (over rows)
  vatt = softmax(cc, axis=t).T      (over cols, transposed)
  out1 = (f1 @ aatt).swap(1,2)      [B,S,S]
  out2 = (f2 @ vatt).swap(1,2)      [B,S,S]

Sharding: pure data parallelism, 2 batches per core on 8 cores; W replicated.

Per core/batch dataflow (all matmuls fp32r = full PE rate, fp32 PSUM accum):
  a1T[e,s] = sum_d W[d,e] f1T[d,s]          (lhsT=W,    rhs=f1T)
  cc [s,t] = sum_e a1T[e,s] f2T[e,t]        (lhsT=a1T,  rhs=f2T)
  vmax[s]  = max_t cc   (DVE free-dim reduce per cc tile, DRAM-bounced to a row)
  ccT[t,s] = sum_e f2T[e,t] a1T[e,s] - vmax[s]
             (K=1 ones x (-vmax row) matmul appended to the accumulation;
              PSUM drain IS the exp -> e2T[t,s] in one ACT op)
  amax[t]  = max_s cc   (DVE max-combine of 8 tiles + 1 gpsimd partition allreduce)
  e1 [u,t] = exp(cc - amax[t])  in place    (DVE sub + ACT exp)
  asum[t]  = sum_u e1   (ones x e1 matmul -> [1,512] PSUM row -> bounce -> 1/asum)
  vsum[s]  = sum_u e2T  (same)
  out1[x,s] = (sum_u e1[u,x] f1T[u,s]) * (1/asum[x])  (scale fused in PSUM drain)
  out2[s,t] = (sum_u e2T[u,s] f2T[u,t]) * (1/vsum[s])

Column-halved stats keep the PE dense: ret matmuls of one half start while the
other half's stats are in flight.
"""

import numpy as np
from contextlib import ExitStack

import concourse.bass as bass
import concourse.tile as tile
from concourse import bacc, mybir, bass_isa
from concourse.bass_utils import run_bass_kernel_spmd

f32 = mybir.dt.float32
f32r = mybir.dt.float32r

P = 128
N = 1024
NT = N // P          # 8 tiles per matrix dim
NB = 2               # batches per core
NCORES = 8
HALF = 512           # matmul moving free dim / psum bank
Exp = mybir.ActivationFunctionType.Exp
Copy = mybir.ActivationFunctionType.Copy


def _build():
    nc = bacc.Bacc("TRN2", target_bir_lowering=False, debug=False, num_devices=NCORES)

    f1t_d = nc.dram_tensor("f1t", [NB, N, N], f32r, kind="ExternalInput").ap()
    f2t_d = nc.dram_tensor("f2t", [NB, N, N], f32r, kind="ExternalInput").ap()
    w_d = nc.dram_tensor("w", [N, N], f32r, kind="ExternalInput").ap()
    o1_d = nc.dram_tensor("o1", [NB, N, N], f32, kind="ExternalOutput").ap()
    o2_d = nc.dram_tensor("o2", [NB, N, N], f32, kind="ExternalOutput").ap()

    with tile.TileContext(nc) as tc, ExitStack() as ctx:
        wp = ctx.enter_context(tc.tile_pool(name="wp", bufs=1))
        f1p = ctx.enter_context(tc.tile_pool(name="f1p", bufs=1))
        f2p = ctx.enter_context(tc.tile_pool(name="f2p", bufs=1))
        a1p = ctx.enter_context(tc.tile_pool(name="a1p", bufs=1))
        ccp = ctx.enter_context(tc.tile_pool(name="ccp", bufs=1))
        cctp = ctx.enter_context(tc.tile_pool(name="cctp", bufs=1))
        statp = ctx.enter_context(tc.tile_pool(name="statp", bufs=1))
        smallp = ctx.enter_context(tc.tile_pool(name="smallp", bufs=1))
        oretp = ctx.enter_context(tc.tile_pool(name="oretp", bufs=2))
        psp = ctx.enter_context(tc.tile_pool(name="psp", bufs=7, space="PSUM"))
        rowpsp = ctx.enter_context(tc.tile_pool(name="rowpsp", bufs=1, space="PSUM"))
        dscrp = ctx.enter_context(tc.tile_pool(name="dscrp", bufs=2, space="DRAM"))

        # constant ones (fp32r) for bias/sum matmuls (memset can't write f32r)
        ones_f32r_ = smallp.tile([1, P], f32, name="ones_f32r_", tag="ones_f32r_")
        nc.vector.memset(ones_f32r_[:], 1.0)
        ones_k1 = smallp.tile([1, P], f32r, name="ones_k1", tag="ones_k1")
        nc.scalar.copy(ones_k1[:], ones_f32r_[:])
        ones_f32c_ = smallp.tile([P, 1], f32, name="ones_f32c_", tag="ones_f32c_")
        nc.vector.memset(ones_f32c_[:], 1.0)
        ones_col = smallp.tile([P, 1], f32r, name="ones_col", tag="ones_col")
        nc.scalar.copy(ones_col[:], ones_f32c_[:])

        for b in range(NB):
            # ---- loads -------------------------------------------------
            ws, f1s, f2s = [], [], []
            for k in range(NT):
                wk = wp.tile([P, N], f32r, name=f"w{b}_{k}", tag=f"w{k}")
                nc.sync.dma_start(wk[:], w_d[k * P:(k + 1) * P, :])
                ws.append(wk)
            for k in range(NT):
                f1k = f1p.tile([P, N], f32r, name=f"f1_{b}_{k}", tag=f"f1{k}")
                nc.sync.dma_start(f1k[:], f1t_d[b, k * P:(k + 1) * P, :])
                f1s.append(f1k)
            for k in range(NT):
                f2k = f2p.tile([P, N], f32r, name=f"f2_{b}_{k}", tag=f"f2{k}")
                nc.sync.dma_start(f2k[:], f2t_d[b, k * P:(k + 1) * P, :])
                f2s.append(f2k)

            def mmgroup(lhs_tiles, rhs_tiles, m, n, drain, tagpfx, extra=None):
                ps = psp.tile([P, HALF], f32, name=f"ps_{tagpfx}", tag="ps")
                for k in range(NT):
                    nc.tensor.matmul(
                        ps[:],
                        lhs_tiles[k][:, m * P:(m + 1) * P],
                        rhs_tiles[k][:, n * HALF:(n + 1) * HALF],
                        start=(k == 0),
                        stop=(k == NT - 1 and extra is None),
                    )
                if extra is not None:
                    extra(ps)
                drain(m, n, ps)

            # ---- a1T[e,s] ----------------------------------------------
            a1s = [a1p.tile([P, N], f32r, name=f"a1_{b}_{m}", tag=f"a1{m}")
                   for m in range(NT)]
            for m in range(NT):
                for n in range(2):
                    mmgroup(ws, f1s, m, n,
                            lambda m_, n_, ps: nc.scalar.copy(
                                a1s[m_][:, n_ * HALF:(n_ + 1) * HALF], ps[:]),
                            "a1")

            # ---- cc[s,t] + vmax ----------------------------------------
            ccs = [ccp.tile([P, N], f32r, name=f"cc_{b}_{m}", tag=f"cc{m}")
                   for m in range(NT)]
            # nvmax_pp[m] = per-partition -max_t of cc tile m (negated vmax, f32r)
            nvmax_pp = [smallp.tile([P, 1], f32r, name=f"nvmax_{b}_{m}", tag=f"vmaxpp{m}")
                        for m in range(NT)]
            scr_v = dscrp.tile([1, N], f32r, name=f"scr_v{b}", tag="scr_v")
            nvrow = statp.tile([1, N], f32r, name=f"nvrow{b}", tag="nvrow")

            def cc_drain(m, n, ps):
                nc.vector.tensor_copy(ccs[m][:, n * HALF:(n + 1) * HALF], ps[:])

            for m in range(NT):
                for n in range(2):
                    mmgroup(a1s, f2s, m, n, cc_drain, "cc")
                nc.vector.tensor_reduce(
                    out=nvmax_pp[m][:], in_=ccs[m][:].bitcast(f32),
                    axis=mybir.AxisListType.X, op=mybir.AluOpType.max, negate=True)
                # bounce this tile's 128 negated maxes into the scratch row
                nc.sync.dma_start(
                    scr_v[0:1, m * P:(m + 1) * P].rearrange("one (p x) -> (one p) x", p=P),
                    nvmax_pp[m][:])
            for h in range(2):
                nc.sync.dma_start(nvrow[0:1, h * HALF:(h + 1) * HALF],
                                  scr_v[0:1, h * HALF:(h + 1) * HALF])

            # ---- ccT[t,s] - vmax[s], exp-drained -> e2T ------------------
            ccts = [cctp.tile([P, N], f32r, name=f"cct_{b}_{m}", tag=f"cct{m}")
                    for m in range(NT)]

            def cct_drain(m, n, ps):
                nc.scalar.activation(ccts[m][:, n * HALF:(n + 1) * HALF], ps[:], Exp)

            for n in range(2):      # n-outer: half-0 groups run while half-1 row bounces
                for m in range(NT):
                    mmgroup(f2s, a1s, m, n, cct_drain, "cct",
                            extra=lambda ps, n_=n: nc.tensor.matmul(
                                ps[:], ones_k1[:],
                                nvrow[0:1, n_ * HALF:(n_ + 1) * HALF],
                                start=False, stop=True))

            # ---- amax + e1 = exp(cc - amax), per column half -------------
            rsa = smallp.tile([P, NT], f32, name=f"rsa{b}", tag="rsa")
            rsv = smallp.tile([P, NT], f32, name=f"rsv{b}", tag="rsv")
            scr_s = dscrp.tile([1, 4 * N], f32, name=f"scr_s{b}", tag="scr_s")

            def bounce_sum(ps_row, col):   # [1,HALF] psum row -> rs[:, col*4...]
                # hop through SBUF (DMA cannot read PSUM); reuse a dead a1 slot
                srow = a1p.tile([1, HALF], f32, name="sumrow", tag="a10")
                nc.scalar.copy(srow[:], ps_row[:])
                nc.sync.dma_start(scr_s[0:1, col * HALF:(col + 1) * HALF], srow[:])

            for h in range(2):
                sl = slice(h * HALF, (h + 1) * HALF)
                amaxt = statp.tile([P, HALF], f32, name=f"amaxt{b}{h}", tag="amaxt")
                nc.vector.tensor_copy(amaxt[:], ccs[0][:, sl].bitcast(f32))
                for m in range(1, NT):
                    nc.vector.tensor_tensor(
                        out=amaxt[:], in0=amaxt[:], in1=ccs[m][:, sl].bitcast(f32),
                        op=mybir.AluOpType.max)
                amaxB = statp.tile([P, HALF], f32, name=f"amaxB{b}{h}", tag="amaxB")
                nc.gpsimd.partition_all_reduce(
                    amaxB[:], amaxt[:], channels=P, reduce_op=bass_isa.ReduceOp.max)
                for m in range(NT):
                    nc.vector.tensor_tensor(
                        out=ccs[m][:, sl], in0=ccs[m][:, sl].bitcast(f32),
                        in1=amaxB[:], op=mybir.AluOpType.subtract)
                    nc.scalar.activation(ccs[m][:, sl], ccs[m][:, sl].bitcast(f32), Exp)

            # ---- asum (over e1) -> rsa ; ret1 ; vsum (over e2T) ; ret2 ---
            def colsum_mm(tiles, h, col):
                sps = rowpsp.tile([1, HALF], f32, name="sps", tag="rowps")
                for k in range(NT):
                    nc.tensor.matmul(
                        sps[:], ones_col[:], tiles[k][:, h * HALF:(h + 1) * HALF],
                        start=(k == 0), stop=(k == NT - 1))
                bounce_sum(sps, col)

            for h in range(2):
                colsum_mm(ccs, h, h)          # asum halves at scr_s cols 0,1
            for h in range(2):
                rd = scr_s[0:1, h * HALF:(h + 1) * HALF].rearrange(
                    "one (m p) -> (one p) m", p=P)
                nc.sync.dma_start(rsa[:, 4 * h:4 * h + 4], rd)
            nc.vector.reciprocal(rsa[:], rsa[:])

            def ret_drain(out_d, rs):
                def d(m, n, ps):
                    ot = oretp.tile([P, HALF], f32, name="oret", tag="oret")
                    nc.scalar.activation(ot[:], ps[:], Copy,
                                         bias=0.0, scale=rs[:, m:m + 1])
                    nc.sync.dma_start(
                        out_d[b, m * P:(m + 1) * P, n * HALF:(n + 1) * HALF], ot[:])
                return d

            for m in range(NT):
                for n in range(2):
                    mmgroup(ccs, f1s, m, n, ret_drain(o1_d, rsa), "r1")

            for h in range(2):
                colsum_mm(ccts, h, 2 + h)     # vsum halves at scr_s cols 2,3
            for h in range(2):
                rd = scr_s[0:1, (2 + h) * HALF:(3 + h) * HALF].rearrange(
                    "one (m p) -> (one p) m", p=P)
                nc.sync.dma_start(rsv[:, 4 * h:4 * h + 4], rd)
            nc.vector.reciprocal(rsv[:], rsv[:])

            for m in range(NT):
                for n in range(2):
                    mmgroup(ccts, f2s, m, n, ret_drain(o2_d, rsv), "r2")

    nc.compile()
    return nc


_NC = None
TRACE = False
LAST = None


def _get_nc():
    global _NC
    if _NC is None:
        _NC = _build()
    return _NC


def kernel(f1_norm, f2_norm, corr_weights):
    f1_norm = np.ascontiguousarray(f1_norm, dtype=np.float32)
    f2_norm = np.ascontiguousarray(f2_norm, dtype=np.float32)
    w = np.ascontiguousarray(corr_weights, dtype=np.float32)
    B = f1_norm.shape[0]
    assert B == NB * NCORES

    # host-side feature-major transposes: f1t[b] = f1[b].T
    f1t = np.ascontiguousarray(np.swapaxes(f1_norm, 1, 2))
    f2t = np.ascontiguousarray(np.swapaxes(f2_norm, 1, 2))

    nc = _get_nc()
    in_maps = [
        {"f1t": f1t[c * NB:(c + 1) * NB], "f2t": f2t[c * NB:(c + 1) * NB], "w": w}
        for c in range(NCORES)
    ]
    res = run_bass_kernel_spmd(nc, in_maps, core_ids=list(range(NCORES)), trace=TRACE)
    global LAST
    LAST = res
    out1 = np.concatenate([res.results[c]["o1"] for c in range(NCORES)], axis=0)
    out2 = np.concatenate([res.results[c]["o2"] for c in range(NCORES)], axis=0)
    return out1, out2
